# revision 1
# baseline (speedup 1.0000x reference)
"""Trainium2 Bass kernel for nn_CrossAttention (dense_transformer).

Strategy: data-parallel over batch B=8 across the 8 NeuronCores (one batch
element per core). Inside each core:

  - LayerNorm stats via bn_stats/bn_aggr (DVE), applied as a fused
    (x - mu) * rstd tensor_scalar on GPSIMD, output bf16.
  - PE transpose to feature-major layout.
  - q/kv projections as bf16 PE matmuls. The LN affine (gamma/beta) and the
    1/sqrt(c) attention scale are folded into the weights on the host.
  - Depthwise 3x3 conv done on the PE: 9 shifted matmuls per channel group
    with 32x32 diagonal weight blocks (tile_position packing), accumulating
    in PSUM.
  - Attention computed transposed (S^T = k_h^T.T @ q_h^T) so softmax's exp
    runs straight out of PSUM on the scalar engine and P^T feeds the PV
    matmul directly. Row sums come from an extra ones-column in V.
    Max-subtraction is skipped: logits are ~N(0, 0.2) here, exp is safe.
  - Final PE transpose back to token-major, with normalization (1/rowsum)
    and the conv-skip add fused into one scalar_tensor_tensor per head.
"""

import numpy as np
import ml_dtypes

import concourse.bass as bass
import concourse.mybir as mybir
import concourse.tile as tile
from concourse import bacc, bass_utils
from concourse.masks import make_identity

F32 = mybir.dt.float32
BF16 = mybir.dt.bfloat16
AF = mybir.ActivationFunctionType
OP = mybir.AluOpType

N_CORES = 8
N1 = 1024          # query tokens (= H*W = 32*32)
N2 = 1024          # key tokens
DIM = 512
NH = 8
CH = 64            # head dim
HH = 32            # H
WW = 32            # W
NTOK = N1 // 128   # 8 token tiles
NCH = DIM // 128   # 4 feature chunks
EPS = 1e-5

# tap order: center first so its start=True write covers every pixel of the
# psum accumulation region before the partial (edge-clipped) taps accumulate.
TAPS = [(0, 0), (-1, -1), (-1, 0), (-1, 1), (0, -1), (0, 1),
        (1, -1), (1, 0), (1, 1)]


def _build_program(trace_sim=False, bench_iters=0, phases="ABCDE"):
    nc = bacc.Bacc("TRN2", target_bir_lowering=False, debug=False,
                   enable_asserts=True, num_devices=N_CORES)

    q_ap = nc.dram_tensor("query", [N1, DIM], F32, kind="ExternalInput").ap()
    k_ap = nc.dram_tensor("key", [N2, DIM], F32, kind="ExternalInput").ap()
    wq_ap = nc.dram_tensor("wq", [NCH, 128, DIM], BF16, kind="ExternalInput").ap()
    wk_ap = nc.dram_tensor("wk", [NCH, 128, DIM], BF16, kind="ExternalInput").ap()
    wv_ap = nc.dram_tensor("wv", [NCH, 128, DIM], BF16, kind="ExternalInput").ap()
    dw_ap = nc.dram_tensor("dw", [NCH, 128, 9 * 32], BF16, kind="ExternalInput").ap()
    bq_ap = nc.dram_tensor("bq", [128, NCH], F32, kind="ExternalInput").ap()
    bk_ap = nc.dram_tensor("bk", [128, NCH], F32, kind="ExternalInput").ap()
    cb_ap = nc.dram_tensor("cb", [128, NCH], F32, kind="ExternalInput").ap()
    out_ap = nc.dram_tensor("out", [N1, DIM], F32, kind="ExternalOutput").ap()

    with tile.TileContext(nc, trace_sim=trace_sim) as tc:
        if bench_iters:
            # benchmark build: run the whole body bench_iters times inside
            # one NEFF so device time is measurable above host/RPC overhead
            with tc.For_i(0, bench_iters, 1):
                _emit(nc, tc, q_ap, k_ap, wq_ap, wk_ap, wv_ap, dw_ap,
                      bq_ap, bk_ap, cb_ap, out_ap, phases)
        else:
            _emit(nc, tc, q_ap, k_ap, wq_ap, wk_ap, wv_ap, dw_ap, bq_ap,
                  bk_ap, cb_ap, out_ap, phases)
    nc.compile()
    return nc


def _emit(nc, tc, q_ap, k_ap, wq_ap, wk_ap, wv_ap, dw_ap, bq_ap, bk_ap,
          cb_ap, out_ap, phases="ABCDE"):
    from contextlib import ExitStack
    ctx = ExitStack()

    const = ctx.enter_context(tc.tile_pool(name="const", bufs=1))

    ident_bf = const.tile([128, 128], BF16, tag="identbf", name="identbf")
    make_identity(nc, ident_bf[:])
    ident_f32 = const.tile([128, 128], F32, tag="identf32", name="identf32")
    make_identity(nc, ident_f32[:])
    eps_t = const.tile([128, 1], F32, tag="eps", name="eps")
    nc.gpsimd.memset(eps_t[:], EPS)

    wq_sb = [const.tile([128, DIM], BF16, tag=f"wq{g}", name=f"wq{g}") for g in range(NCH)]
    wk_sb = [const.tile([128, DIM], BF16, tag=f"wk{g}", name=f"wk{g}") for g in range(NCH)]
    wv_sb = [const.tile([128, DIM], BF16, tag=f"wv{g}", name=f"wv{g}") for g in range(NCH)]
    dw_sb = [const.tile([128, 9 * 32], BF16, tag=f"dw{g}", name=f"dw{g}") for g in range(NCH)]
    for g in range(NCH):
        nc.sync.dma_start(wq_sb[g][:], wq_ap[g])
        nc.gpsimd.dma_start(wk_sb[g][:], wk_ap[g])
        nc.sync.dma_start(wv_sb[g][:], wv_ap[g])
        nc.gpsimd.dma_start(dw_sb[g][:], dw_ap[g])
    bq_sb = const.tile([128, NCH], F32, tag="bq", name="bq")
    bk_sb = const.tile([128, NCH], F32, tag="bk", name="bk")
    cb_sb = const.tile([128, NCH], F32, tag="cb", name="cb")
    nc.sync.dma_start(bq_sb[:], bq_ap)
    nc.sync.dma_start(bk_sb[:], bk_ap)
    nc.sync.dma_start(cb_sb[:], cb_ap)

    persist = ctx.enter_context(tc.tile_pool(name="persist", bufs=1))

    # ---- Phase A: load + layernorm + transpose (query and key) ----------
    # produces lnT[g] = LN(x)^T  [128 feat, 1024 tok] bf16 per feature chunk
    lnqT = [persist.tile([128, N1], BF16, tag=f"lnqT{g}", name=f"lnqT{g}") for g in range(NCH)]
    lnkT = [persist.tile([128, N2], BF16, tag=f"lnkT{g}", name=f"lnkT{g}") for g in range(NCH)]

    with tc.tile_pool(name="ln_work", bufs=1) as work, \
         tc.tile_pool(name="ln_psum", bufs=1, space="PSUM") as psA:
        for src_ap, lnT in (((q_ap, lnqT), (k_ap, lnkT)) if "A" in phases else ()):
            for half in range(2):
                psts = [psA.tile([128, 512], BF16, tag=f"pst{g}", name=f"pst{g}")
                        for g in range(NCH)]
                for ii in range(4):
                    i = half * 4 + ii
                    xt = work.tile([128, DIM], F32, tag="xin", bufs=8, name="xin")
                    eng = nc.sync if i % 2 == 0 else nc.gpsimd
                    eng.dma_start(xt[:], src_ap[i * 128:(i + 1) * 128, :])
                    bn6 = work.tile([128, 6], F32, tag="bn6", bufs=4, name="bn6")
                    nc.vector.bn_stats(out=bn6[:], in_=xt[:])
                    mv = work.tile([128, 2], F32, tag="mv", bufs=4, name="mv")
                    nc.vector.bn_aggr(out=mv[:], in_=bn6[:])
                    rstd = work.tile([128, 1], F32, tag="rstd", bufs=4, name="rstd")
                    nc.scalar.activation(out=rstd[:], in_=mv[:, 1:2],
                                         func=AF.Sqrt, bias=eps_t[:], scale=1.0)
                    nc.vector.reciprocal(out=rstd[:], in_=rstd[:])
                    ln = work.tile([128, DIM], BF16, tag="ln", bufs=4, name="ln")
                    nc.vector.tensor_scalar(
                        out=ln[:], in0=xt[:],
                        scalar1=mv[:, 0:1], scalar2=rstd[:],
                        op0=OP.subtract, op1=OP.mult)
                    for g in range(NCH):
                        nc.tensor.transpose(psts[g][:, ii * 128:ii * 128 + 128],
                                            ln[:, g * 128:(g + 1) * 128],
                                            ident_bf[:])
                for g in range(NCH):
                    nc.vector.tensor_copy(
                        out=lnT[g][:, half * 512:(half + 1) * 512],
                        in_=psts[g][:])

    # ---- Phase B: projections -------------------------------------------
    # qT[g] = (LNq @ Wq')^T   [128, 1024] bf16 (scaled by 1/8, bias folded)
    # kT[g] = (LNk @ Wk')^T   [128, 1024] bf16
    # v_aug[i] = [v | 1]      [128 tok, 8*65] bf16 token-major
    qT = [persist.tile([128, N1], BF16, tag=f"qT{g}", name=f"qT{g}") for g in range(NCH)]
    kT = [persist.tile([128, N2], BF16, tag=f"kT{g}", name=f"kT{g}") for g in range(NCH)]
    v_aug = [persist.tile([128, NH * (CH + 1)], BF16, tag=f"vaug{i}", name=f"vaug{i}")
             for i in range(NTOK)]

    with tc.tile_pool(name="proj_psum", bufs=4, space="PSUM") as psB:
        for lnT, w_sb, b_sb, bi, dstT in (((lnqT, wq_sb, bq_sb, 0, qT),
                                           (lnkT, wk_sb, bk_sb, 1, kT))
                                          if "B" in phases else ()):
            for m in range(NCH):          # output feature chunk
                for half in range(2):     # token half
                    ps = psB.tile([128, 512], F32, tag="proj", name="proj")
                    for kc in range(NCH):
                        nc.tensor.matmul(
                            ps[:], w_sb[kc][:, m * 128:(m + 1) * 128],
                            lnT[kc][:, half * 512:(half + 1) * 512],
                            start=(kc == 0), stop=(kc == NCH - 1))
                    # evacuate on ACT (idle until the first exp) to keep DVE
                    # free; Identity applies the per-partition bias
                    nc.scalar.activation(
                        out=dstT[m][:, half * 512:(half + 1) * 512],
                        in_=ps[:], func=AF.Identity,
                        bias=b_sb[:, m:m + 1], scale=1.0)
        for i in (range(NTOK) if "B" in phases else ()):
            # ones column for the softmax row-sum trick; data columns are
            # fully overwritten by the evac copy below, so no memset needed
            nc.vector.tensor_scalar(
                out=v_aug[i][:].rearrange("p (h c) -> p h c", c=CH + 1)[:, :, CH],
                in0=ident_f32[:, 0:NH], scalar1=0.0, scalar2=1.0,
                op0=OP.mult, op1=OP.add)
            ps = psB.tile([128, 512], F32, tag="proj", name="proj")
            for kc in range(NCH):
                nc.tensor.matmul(
                    ps[:], lnkT[kc][:, i * 128:(i + 1) * 128], wv_sb[kc][:],
                    start=(kc == 0), stop=(kc == NCH - 1))
            nc.vector.tensor_copy(
                out=v_aug[i][:].rearrange("p (h c) -> p h c", c=CH + 1)[:, :, 0:CH],
                in_=ps[:].rearrange("p (h c) -> p h c", c=CH))

    skipT = [persist.tile([128, N1], F32, tag=f"skipT{g}", name=f"skipT{g}") for g in range(NCH)]
    attn = [persist.tile([CH + 1, N1], F32, tag=f"attn{h}", name=f"attn{h}") for h in range(NH)]
    PW = WW + 2
    # ---- Phase C: depthwise conv on PE (own psum pool, emitted after the
    # attention pairs so its matmuls fill PE gaps during the exp stream) ---
    with tc.tile_pool(name="conv_psum", bufs=2, space="PSUM") as psC:
        for g in (range(NCH) if "C" in phases else ()):
            qTp = persist.tile([128, PW * PW], BF16, tag=f"qTp{g}",
                               name=f"qTp{g}")
            qTp3 = qTp[:].rearrange("p (y x) -> p y x", x=PW)
            # zero only the pad border (DVE affine; gpsimd memset is slow)
            for view, w in ((qTp3[:, 0, :], PW), (qTp3[:, PW - 1, :], PW),
                            (qTp3[:, 1:PW - 1, 0], PW - 2),
                            (qTp3[:, 1:PW - 1, PW - 1], PW - 2)):
                nc.vector.tensor_scalar(
                    out=view, in0=ident_f32[:, 0:w], scalar1=0.0,
                    scalar2=None, op0=OP.mult)
            nc.vector.tensor_copy(
                out=qTp3[:, 1:HH + 1, 1:WW + 1],
                in_=qT[g][:].rearrange("p (y x) -> p y x", x=WW))
            cps = psC.tile([128, N1], F32, tag="conv", name="conv")
            for yh in range(2):
                for t, (dy, dx) in enumerate(TAPS):
                    y0 = yh * 16 + 1 + dy
                    for i in range(4):
                        nc.tensor.matmul(
                            cps[32 * i:32 * i + 32,
                                yh * 512:yh * 512 + 512],
                            dw_sb[g][32 * i:32 * i + 32, t * 32:t * 32 + 32],
                            qTp3[32 * i:32 * i + 32, y0:y0 + 16,
                                 1 + dx:1 + dx + WW],
                            start=(t == 0), stop=(t == len(TAPS) - 1),
                            tile_position=(32 * i, 32 * i),
                            skip_group_check=True)
            nc.vector.tensor_scalar(
                out=skipT[g][:], in0=cps[:], scalar1=cb_sb[:, g:g + 1],
                scalar2=None, op0=OP.add)

    # ---- Phase D: attention (emitted before conv so exp starts ASAP) ----
    # S^T_h tile j: [128 tk, 1024 tq] = k_h^T(chunk j).T @ q_h^T
    # P^T = exp(S^T); PV: out^T_aug[65, 1024] = v_aug_h.T @ P^T

    # ---- Phase C: depthwise conv on PE ----------------------------------
    # skipT[g][c, y*32+x] = sum_taps w8[c,tap] * qT[g][c, (y+dy)*32+(x+dx)]
    # Input is copied into a zero-padded 34x34 image so every tap reads a
    # shifted 3D window and writes a contiguous 2D psum region. The 4
    # channel sub-blocks run concurrently in the PE via diagonal
    # tile_position packing.
    # ---- Phase D: attention, then C: conv (conv emitted later so its PE
    # matmuls fill the gaps while ACT chews through the exps) --------------
    # Heads are processed in even/odd pairs occupying PE row groups 0-63 and
    # 64-127 so their K=64 QK matmuls run concurrently in the array.
    with tc.tile_pool(name="st_psum", bufs=3, space="PSUM") as psST, \
         tc.tile_pool(name="pv_psum", bufs=1, space="PSUM") as psPV, \
         tc.tile_pool(name="pT_pool", bufs=20) as pTp:
        for g in (range(NCH) if "D" in phases else ()):  # head pair
            pts = {0: [], 1: []}
            for j in range(NTOK):
                for r_i in range(2):
                    r = r_i * CH
                    st = psST.tile([128, N1], F32, tag="st", name="st")
                    for half in range(2):
                        nc.tensor.matmul(
                            st[:, half * 512:(half + 1) * 512],
                            kT[g][r:r + CH, j * 128:(j + 1) * 128],
                            qT[g][r:r + CH, half * 512:(half + 1) * 512],
                            start=True, stop=True, tile_position=(r, 0))
                    pt = pTp.tile([128, N1], BF16, tag="pt", name="pt")
                    nc.scalar.activation(out=pt[:], in_=st[:], func=AF.Exp)
                    pts[r_i].append(pt)
            for r_i in range(2):
                h = 2 * g + r_i
                pv = psPV.tile([CH + 1, N1], F32, tag="pv", name="pv")
                for j in range(NTOK):
                    for half in range(2):
                        nc.tensor.matmul(
                            pv[:, half * 512:(half + 1) * 512],
                            v_aug[j][:, h * (CH + 1):(h + 1) * (CH + 1)],
                            pts[r_i][j][:, half * 512:(half + 1) * 512],
                            start=(j == 0), stop=(j == NTOK - 1))
                nc.vector.tensor_copy(out=attn[h][:], in_=pv[:])

    # ---- Phase E: transpose back to token-major, normalize, add skip ----
    recip = persist.tile([128, NTOK * NH], F32, tag="recip", name="recip")
    with tc.tile_pool(name="tok_psum", bufs=2, space="PSUM") as psTok, \
         tc.tile_pool(name="skip_psum", bufs=2, space="PSUM") as psSk, \
         tc.tile_pool(name="fin_pool", bufs=3) as finp:
        for tb in (range(NTOK) if "E" in phases else ()):
            sps = psSk.tile([128, DIM], F32, tag="skps", name="skps")
            for g in range(NCH):
                nc.tensor.transpose(sps[:, g * 128:(g + 1) * 128],
                                    skipT[g][:, tb * 128:(tb + 1) * 128],
                                    ident_f32[:])
            sk_tok = finp.tile([128, DIM], F32, tag="sktok", name="sktok")
            nc.vector.tensor_copy(out=sk_tok[:], in_=sps[:])

            tps_ab = [psTok.tile([128, 4 * (CH + 1)], F32, tag=f"tokps{s}",
                                 name=f"tokps{s}") for s in range(2)]
            for h in range(NH):
                tps = tps_ab[h // 4]
                nc.tensor.transpose(
                    tps[:, (h % 4) * (CH + 1):(h % 4 + 1) * (CH + 1)],
                    attn[h][:, tb * 128:(tb + 1) * 128],
                    ident_f32[0:CH + 1, 0:CH + 1])
            tps3_ab = [t[:].rearrange("p (h c) -> p h c", c=CH + 1)
                       for t in tps_ab]
            for s in range(2):
                nc.vector.reciprocal(
                    out=recip[:, tb * NH + 4 * s:tb * NH + 4 * s + 4],
                    in_=tps3_ab[s][:, :, CH])
            fin = finp.tile([128, DIM], F32, tag="fin", name="fin")
            for h in range(NH):
                nc.vector.scalar_tensor_tensor(
                    out=fin[:, h * CH:(h + 1) * CH],
                    in0=tps3_ab[h // 4][:, h % 4, 0:CH],
                    scalar=recip[:, tb * NH + h:tb * NH + h + 1],
                    in1=sk_tok[:, h * CH:(h + 1) * CH],
                    op0=OP.mult, op1=OP.add)
            nc.sync.dma_start(out_ap[tb * 128:(tb + 1) * 128, :], fin[:])

    ctx.close()


_CACHE = {}


def _get_runner():
    """Build the program once and wrap it in a reusable jitted SPMD callable.

    run_bass_kernel_spmd re-traces a fresh closure on every call; caching the
    jitted shard_map keeps steady-state calls at PJRT-execute cost only.
    """
    if "runner" in _CACHE:
        return _CACHE["runner"]

    import jax
    from jax.sharding import Mesh, PartitionSpec
    from jax.experimental.shard_map import shard_map
    from concourse import bass2jax
    import concourse.mybir as mb

    nc = _build_program()
    bass2jax.install_neuronx_cc_hook()

    part_name = (nc.partition_id_tensor.name
                 if nc.partition_id_tensor else None)
    in_names, out_names, out_avals = [], [], []
    for alloc in nc.m.functions[0].allocations:
        if not isinstance(alloc, mb.MemoryLocationSet):
            continue
        name = alloc.memorylocations[0].name
        if alloc.kind == "ExternalInput":
            if name != part_name:
                in_names.append(name)
        elif alloc.kind == "ExternalOutput":
            out_names.append(name)
            out_avals.append(jax.core.ShapedArray(
                tuple(alloc.tensor_shape), mb.dt.np(alloc.dtype)))
    n_params = len(in_names)
    all_names = in_names + out_names
    if part_name is not None:
        all_names = all_names + [part_name]

    def _body(*args):
        operands = list(args)
        if part_name is not None:
            operands.append(bass2jax.partition_id_tensor())
        outs = bass2jax._bass_exec_p.bind(
            *operands,
            out_avals=tuple(out_avals),
            in_names=tuple(all_names),
            out_names=tuple(out_names),
            lowering_input_output_aliases=(),
            sim_require_finite=True,
            sim_require_nnan=True,
            nc=nc,
        )
        return tuple(outs)

    devices = jax.devices()[:N_CORES]
    mesh = Mesh(np.asarray(devices), ("core",))
    n_outs = len(out_names)
    sharded = jax.jit(
        shard_map(_body, mesh=mesh,
                  in_specs=(PartitionSpec("core"),) * (n_params + n_outs),
                  out_specs=(PartitionSpec("core"),) * n_outs,
                  check_rep=False),
        donate_argnums=tuple(range(n_params, n_params + n_outs)),
        keep_unused=True)

    from jax.sharding import NamedSharding
    import jax.numpy as jnp

    zero_shard = NamedSharding(mesh, PartitionSpec("core"))
    make_zeros = jax.jit(
        lambda: tuple(jnp.zeros((N_CORES * a.shape[0], *a.shape[1:]), a.dtype)
                      for a in out_avals),
        out_shardings=(zero_shard,) * len(out_avals))
    dev_cache = {}

    import hashlib

    def run(in_maps):
        concat_in = []
        for name in in_names:
            same = all(in_maps[c][name] is in_maps[0][name]
                       for c in range(N_CORES))
            if same:
                # replicated constants (weights): keep device-resident,
                # keyed by content hash so changed weights re-upload
                key = (name,
                       hashlib.sha1(np.ascontiguousarray(
                           in_maps[0][name]).tobytes()).hexdigest())
                if key not in dev_cache:
                    arr = np.concatenate(
                        [np.asarray(in_maps[c][name])
                         for c in range(N_CORES)], axis=0)
                    dev_cache[key] = jax.device_put(arr, zero_shard)
                concat_in.append(dev_cache[key])
                continue
            concat_in.append(np.concatenate(
                [np.asarray(in_maps[c][name]) for c in range(N_CORES)],
                axis=0))
        out_arrs = sharded(*concat_in, *make_zeros())
        return [
            {name: np.asarray(out_arrs[i]).reshape(
                N_CORES, *out_avals[i].shape)[c]
             for i, name in enumerate(out_names)}
            for c in range(N_CORES)]

    _CACHE["runner"] = run
    return run


def _prepare_in_maps(query, key, gq, bq_ln, gk, bk_ln, Wq, bq, Wkv, bkv,
                     conv_w, conv_b, H, W):
    query = np.asarray(query, np.float32)
    key = np.asarray(key, np.float32)
    gq = np.asarray(gq, np.float32); bq_ln = np.asarray(bq_ln, np.float32)
    gk = np.asarray(gk, np.float32); bk_ln = np.asarray(bk_ln, np.float32)
    Wq = np.asarray(Wq, np.float32); bq = np.asarray(bq, np.float32)
    Wkv = np.asarray(Wkv, np.float32); bkv = np.asarray(bkv, np.float32)
    conv_w = np.asarray(conv_w, np.float32)
    conv_b = np.asarray(conv_b, np.float32)
    assert int(H) == HH and int(W) == WW
    B, n1, dim_q = query.shape
    assert (B, n1, dim_q) == (N_CORES, N1, DIM) and key.shape == (N_CORES, N2, DIM)

    scale = (DIM // NH) ** (-0.5)
    # fold LN affine + attention scale into the q projection; the depthwise
    # conv weights absorb the inverse scale (conv is linear in q).
    wq_pre = (gq[:, None] * Wq) * scale
    bq_pre = (bq_ln @ Wq + bq) * scale
    wkv_pre = gk[:, None] * Wkv
    bkv_pre = bk_ln @ Wkv + bkv
    wk_pre, wv_pre = wkv_pre[:, :DIM], wkv_pre[:, DIM:]
    bk_pre, bv_pre = bkv_pre[:DIM], bkv_pre[DIM:]
    # v-bias: softmax weights sum to 1, so +bv on v == +bv on the output;
    # fold it into the (per-channel) conv bias which is added at the end.
    cb_pre = conv_b + bv_pre

    w8 = conv_w[:, 0, :, :] / scale  # [512, 3, 3]
    dw = np.zeros((NCH, 128, 9 * 32), np.float32)
    for t, (dy, dx) in enumerate(TAPS):
        wt = w8[:, dy + 1, dx + 1].reshape(NCH, 128)
        for g in range(NCH):
            c = np.arange(128)
            dw[g, c, t * 32 + (c % 32)] = wt[g]

    bf = ml_dtypes.bfloat16
    common = {
        "wq": np.ascontiguousarray(wq_pre.reshape(NCH, 128, DIM)).astype(bf),
        "wk": np.ascontiguousarray(wk_pre.reshape(NCH, 128, DIM)).astype(bf),
        "wv": np.ascontiguousarray(wv_pre.reshape(NCH, 128, DIM)).astype(bf),
        "dw": dw.astype(bf),
        "bq": np.ascontiguousarray(bq_pre.reshape(NCH, 128).T),
        "bk": np.ascontiguousarray(bk_pre.reshape(NCH, 128).T),
        "cb": np.ascontiguousarray(cb_pre.reshape(NCH, 128).T),
    }
    return [dict(common, query=np.ascontiguousarray(query[c]),
                 key=np.ascontiguousarray(key[c])) for c in range(N_CORES)]


def kernel(**inputs):
    in_maps = _prepare_in_maps(**inputs)
    run = _get_runner()
    results = run(in_maps)
    return np.stack([results[c]["out"] for c in range(N_CORES)], axis=0)

